# revision 37
# baseline (speedup 1.0000x reference)
"""DEMA (double exponential moving average) Trainium2 kernel — blocked FIR.

Math: the per-lane recurrence
    s_t = a*x_t + (1-a)*(s_{t-1} + b_{t-1})
    b_t = B*(s_t - s_{t-1}) + (1-B)*b_{t-1}
is linear time-invariant with spectral radius sqrt(1-a) ~ 0.837, so the
impulse response decays below 1e-9 within 128 steps.  The scan is therefore
computed as a TRUNCATED FIR over two 128-step blocks: for output block c,
    out_c = H0 @ X_c + H1 @ X_{c-1}           (c >= 2)
    out_1 = H0 @ X_1 + G1 @ X_0               (G1 carries the s0/b0 init)
    out_0 = G0 @ X_0                          (row 0 of G0 is identity: s_0=x_0)
There is NO cross-block state and NO serial chain: every block is two
accumulating 128x128 @ 128x512 matmuls into one PSUM bank.

Numerics: input, weights and output are bf16 (fp32 PSUM accumulation),
halving HBM traffic.  Simulated end-to-end rel err vs the fp32 reference is
~3e-3 (tolerance 2e-2); fp32 truncation error of the 256-tap FIR is ~1e-7.

Sharding: pure data parallel over batch, 4 batches per core x 8 cores.
The host casts x to bf16, pre-swizzles it into the on-chip tile layout
([group, partition, chunk, feat]) so every DMA descriptor moves 8 KiB of
contiguous HBM, and inverts the swizzle + casts back to fp32 on the result.

Engines: SP ring = input DMAs (1 MiB, 8 chunks per transfer), PE = matmuls,
DVE/ACT split the paired two-bank PSUM->SBUF output copies, ACT issues the
output DMAs.  Synchronization is explicit single-wait semaphores (at most
one sem wait per instruction on this toolchain).  DMA completion sems are
per buffer-slot so at most one group's DMAs are in flight per sem.
"""

import math
from contextlib import ExitStack

import numpy as np
import ml_dtypes

import concourse.bass as bass
from concourse import mybir
from concourse.bass_utils import run_bass_kernel_spmd

ALPHA = 0.3
BETA = 0.1

B, T, F = 32, 4096, 512
NCORES = 8
BLOC = B // NCORES
K = 128           # timesteps per block = matmul contraction/output size
NCH = T // K      # 32 blocks
GRP = 8           # blocks per grouped DMA (1 MiB bf16)
NGRP = NCH // GRP  # 4 groups
NSLOT = 3         # input buffer slots (only the last group is gated)

BF16 = mybir.dt.bfloat16
F32 = mybir.dt.float32
NP_BF16 = ml_dtypes.bfloat16

# which batches copy PSUM->SBUF on the vector engine (rest use scalar/ACT,
# which also issues the output DMAs)
DVE_COPY_BATCHES = (0, 1)


def _build_mats():
    """Return (G0, G1, H1, H0) float64 [128,128]: out_c = H0@X_c + H1@X_{c-1},
    with G0/G1 handling block 0/1 init (s_0 = x_0, b_0 = x_1 - x_0)."""

    def scan(x):
        # exact reference recurrence, float64, x: [T, n] -> s_t rows incl s_0
        s = x[0].copy()
        b = x[1] - x[0]
        out = [s.copy()]
        for t in range(1, x.shape[0]):
            s_new = ALPHA * x[t] + (1 - ALPHA) * (s + b)
            b = BETA * (s_new - s) + (1 - BETA) * b
            s = s_new
            out.append(s.copy())
        return np.array(out)

    # init-aware columns: impulse at j for j in [0,128)
    imp = np.zeros((2 * K, K))
    imp[:K, :K] = np.eye(K)
    cols = scan(imp)  # [2K, K]
    G0, G1 = cols[:K], cols[K:]

    # steady impulse response (no init effects): h[k] = response at lag k
    x = np.zeros((4 * K, 1))
    J = 2 * K
    x[J, 0] = 1.0
    col = scan(x)[:, 0]
    h = col[J : J + 2 * K]
    idx_i = np.arange(K)[:, None]
    idx_j = np.arange(K)[None, :]
    lag = idx_i - idx_j
    H0 = np.where(lag >= 0, h[np.clip(lag, 0, 2 * K - 1)], 0.0)
    H1 = h[K + lag]
    return G0, G1, H1, H0


def build_nc(bloc=BLOC, t=T, f=F):
    nc = bass.Bass(enable_partition_id=False)
    st = ExitStack()
    nc._dema_exitstack = st  # keep sbuf/psum allocations alive

    nch = t // K
    ngrp = nch // GRP
    gcols = GRP * bloc * f  # free columns per group tile (chunk-major, batch)

    # x/out are host-swizzled to the on-chip tile layout [g, p, (j, b, f)]:
    # element (p, j, b) holds timestep g*GRP*K + j*K + p of batch b, so each
    # group moves as ONE 4 MiB DMA with 32 KiB contiguous per partition.
    x = nc.dram_tensor("x", [ngrp, 128, gcols], BF16, kind="ExternalInput")
    # weights partition-major and flat: one contiguous-1KiB-per-partition DMA
    gw = nc.dram_tensor("gw", [128, 4 * 128], BF16, kind="ExternalInput")
    out = nc.dram_tensor("out", [ngrp, 128, gcols], BF16, kind="ExternalOutput")

    ent = st.enter_context
    wt = ent(nc.sbuf_tensor("wt", [128, 4 * 128], BF16))
    # never-written scratch for PE warm-up matmuls (values irrelevant)
    scr = ent(nc.sbuf_tensor("scr", [128, 512], BF16))
    grhs = [ent(nc.sbuf_tensor(f"grhs_{s}", [128, gcols], BF16)) for s in range(NSLOT)]
    gout = [ent(nc.sbuf_tensor(f"gout_{s}", [128, gcols], BF16)) for s in range(2)]
    ps = [
        [ent(nc.psum_tensor(f"ps{b}_{p}", [128, f], F32)) for p in range(2)]
        for b in range(bloc)
    ]

    # one semaphore per in-flight DMA piece: concurrent DMAs on a shared sem
    # complete UNORDERED across the 16 SDMA engines, so a partial-value wait
    # (e.g. "first quarter landed") is only sound with a dedicated sem.
    NPIECE = 5  # slot 0 carries group 0 as 5 pieces; others use 4
    s_w = nc.alloc_semaphore("s_w")
    s_in = [
        [
            nc.alloc_semaphore(f"s_in{s}_{k}")
            for k in range(NPIECE if s == 0 else 4)
        ]
        for s in range(NSLOT)
    ]
    s_mm = [nc.alloc_semaphore(f"s_mm{b}") for b in range(bloc)]
    s_cp = [nc.alloc_semaphore(f"s_cp{b}") for b in range(bloc)]
    s_out = [nc.alloc_semaphore(f"s_out{s}") for s in range(2)]

    sp, pe, dve, act, pool = nc.sync, nc.tensor, nc.vector, nc.scalar, nc.gpsimd

    all_sems = [s_w] + [s for sl in s_in for s in sl] + s_mm + s_cp + s_out
    sem_nums = sorted(s.num for s in all_sems)
    lo, hi = sem_nums[0], sem_nums[-1] + 1
    assert sem_nums == list(range(lo, hi))

    # Semaphores and DGE sem-tracking are per-core hardware state and are not
    # cleared by allocation: reset ours before any use, and again on exit so
    # repeated executions of this NEFF (and later kernels) see clean state.
    # (A cheaper per-engine sem_clear preamble without dma_reset corrupted
    # the SECOND back-to-back execution — do not remove the entry reset.)
    pool.dma_reset(range(lo, hi))
    pool.sem_clear(range(lo, hi))
    nc.all_engine_barrier()

    in_val = [[0] * len(s_in[s]) for s in range(NSLOT)]  # per piece-sem value
    in_need_q = {}  # (g, j) -> (sem, value) covering chunk-wave j
    in_waited = {}  # sem.num -> highest value PE has already waited for
    out_val = [0, 0]

    def col(j, b):
        return (j * bloc + b) * f

    # ---- weight + input DMAs; weights on the SP ring concurrently with the
    # first two input eighths on the ACT ring (idle this early), so weights
    # and chunk-0 data land together.  Groups stream as four 1 MiB
    # quarter-DMAs (keeps both DMA rings densely populated).
    sp.dma_start(wt[:, :], gw[:, :]).then_inc(s_w, 16)
    nq = 4
    qc = gcols // nq

    wcols = bloc * f  # columns per chunk-wave

    def issue_in_group(g, pieces=None, engines=None):
        slot = g % NSLOT
        if g >= NSLOT:
            for b in range(bloc):
                # slot tiles fully consumed once block (g-2)*GRP is done
                sp.wait_ge(s_mm[b], (g - 2) * GRP + 1)
        if pieces is None:
            pieces = [(q * qc, (q + 1) * qc) for q in range(nq)]
        jdone = 0
        for k, (c0, c1) in enumerate(pieces):
            sem = s_in[slot][k]
            eng = engines[k] if engines else sp
            eng.dma_start(grhs[slot][:, c0:c1], x[g, :, c0:c1]).then_inc(sem, 16)
            in_val[slot][k] += 16
            while (jdone + 1) * wcols <= c1:
                in_need_q[(g, jdone)] = (sem, in_val[slot][k])
                jdone += 1
        assert jdone == GRP

    issue_in_group(
        0,
        pieces=[(0, wcols), (wcols, qc)]
        + [(q * qc, (q + 1) * qc) for q in (1, 2, 3)],
        engines=[act, act, sp, sp, sp],
    )
    for g in range(1, NSLOT):
        issue_in_group(g)

    # ---- PE warm-up: dummy matmuls on a never-written scratch tile (no
    # dependencies, so they start right after the preamble) to lift the HAM
    # clock gate before the real work; results are discarded via start=True.
    for _ in range(6):
        pe.matmul(ps[0][0][:, :], scr[:, 0:128], scr[:, :], start=True, stop=True)
    pe.wait_ge(s_w, 16)

    # ---- main loop over blocks ----
    for cc in range(nch):
        g, j = cc // GRP, cc % GRP
        slot = g % NSLOT
        par = cc % 2  # PSUM bank parity

        # start-of-group bookkeeping: prefetch NSLOT groups ahead; the LAST
        # group's final quarter arrives as two eighths so the tail chain
        # (last input -> compute -> last output) is shorter
        if j == 0 and g + NSLOT < ngrp:
            if g + NSLOT == ngrp - 1:
                issue_in_group(
                    g + NSLOT,
                    pieces=[(q * qc, (q + 1) * qc) for q in (0, 1, 2)]
                    + [(3 * qc, 3 * qc + wcols), (3 * qc + wcols, gcols)],
                )
            else:
                issue_in_group(g + NSLOT)

        # input availability (whole wave shares the tile), per-piece sems.
        # The H1 operand (previous chunk) is always covered: its wave's wait
        # was emitted earlier on this same in-order engine.
        sem, need = in_need_q[(g, j)]
        if in_waited.get(sem.num, -1) < need:
            pe.wait_ge(sem, need)
            in_waited[sem.num] = need

        # matmuls: start pass (G0/G1/H1 weights), then stop pass (H0)
        for b in range(bloc):
            bank = ps[b][par][:, :]
            if cc >= 2:
                pe.wait_ge(s_cp[b], cc - 1)  # bank free (copy of cc-2 done)
            if cc == 0:
                pe.matmul(
                    bank, wt[:, 0:128], grhs[0][:, col(0, b) : col(0, b) + f],
                    start=True, stop=True,
                ).then_inc(s_mm[b], 1)
            else:
                wk = 1 if cc == 1 else 2  # G1 for block 1, else H1
                pj = (cc - 1) % GRP
                pslot = ((cc - 1) // GRP) % NSLOT
                pe.matmul(
                    bank,
                    wt[:, 128 * wk : 128 * (wk + 1)],
                    grhs[pslot][:, col(pj, b) : col(pj, b) + f],
                    start=True, stop=False,
                )
        if cc > 0:
            for b in range(bloc):
                pe.matmul(
                    ps[b][par][:, :],
                    wt[:, 384:512],
                    grhs[slot][:, col(j, b) : col(j, b) + f],
                    start=False, stop=True,
                ).then_inc(s_mm[b], 1)

        # PSUM -> SBUF copies (cast fp32 -> bf16)
        oslot = g % 2
        for b in range(bloc):
            ce = dve if b in DVE_COPY_BATCHES else act
            ce.wait_ge(s_mm[b], cc + 1)
            if j == 0 and out_val[oslot]:
                ce.wait_ge(s_out[oslot], out_val[oslot])
            dst = gout[oslot][:, col(j, b) : col(j, b) + f]
            if ce is act:
                ce.copy(dst, ps[b][par][:, :]).then_inc(s_cp[b], 1)
            else:
                ce.tensor_copy(dst, ps[b][par][:, :]).then_inc(s_cp[b], 1)

        # output drain to HBM (ACT ring): one quarter-DMA per two chunk-waves;
        # the very last quarter drains as two eighths to shorten the tail
        last_grp = g == ngrp - 1
        if last_grp and j >= GRP - 2:
            for b in range(bloc):
                act.wait_ge(s_cp[b], GRP * g + j + 1)
            c0 = 3 * qc + (j - (GRP - 2)) * wcols
            act.dma_start(
                out[g, :, c0 : c0 + wcols], gout[oslot][:, c0 : c0 + wcols]
            ).then_inc(s_out[oslot], 16)
            out_val[oslot] += 16
        elif j % 2 == 1 and not (last_grp and j == GRP - 1):
            q = j // 2
            for b in range(bloc):
                act.wait_ge(s_cp[b], GRP * g + 2 * (q + 1))
            act.dma_start(
                out[g, :, q * qc : (q + 1) * qc], gout[oslot][:, q * qc : (q + 1) * qc]
            ).then_inc(s_out[oslot], 16)
            out_val[oslot] += 16

    # ---- final: ensure all output DMAs land before program end ----
    for slot in range(2):
        if out_val[slot]:
            pool.wait_ge(s_out[slot], out_val[slot])

    # leave semaphores clean for the next load/execution
    pool.dma_reset(range(lo, hi))
    pool.sem_clear(range(lo, hi))

    return nc


_CACHE = {}


def _get_nc():
    if "nc" not in _CACHE:
        _CACHE["nc"] = build_nc()
    return _CACHE["nc"]


def _get_gw():
    if "gw" not in _CACHE:
        g0, g1, h1, h0 = _build_mats()
        # matmul computes lhsT.T @ rhs, so store transposed matrices; laid
        # out partition-major and flattened to [p, 4*128]
        _CACHE["gw"] = np.ascontiguousarray(
            np.stack([g0.T, g1.T, h1.T, h0.T])
            .transpose(1, 0, 2)
            .reshape(128, 4 * 128)
            .astype(NP_BF16)
        )
    return _CACHE["gw"]


def _swizzle(xc):
    """[bloc, T, F] -> [ngrp, 128, GRP*bloc*F] tile layout (see build_nc)."""
    b = xc.shape[0]
    return np.ascontiguousarray(
        xc.reshape(b, NGRP, GRP, 128, F).transpose(1, 3, 2, 0, 4)
    ).reshape(NGRP, 128, GRP * b * F)


def _unswizzle(oc):
    """Inverse of _swizzle: [ngrp, 128, GRP*bloc*F] -> [bloc, T, F]."""
    return (
        oc.reshape(NGRP, 128, GRP, BLOC, F)
        .transpose(3, 0, 2, 1, 4)
        .reshape(BLOC, T, F)
    )


def _run(x, **kwargs):
    x = np.asarray(x)
    assert x.shape == (B, T, F), x.shape
    nc = _get_nc()
    gwv = _get_gw()
    xb = x.astype(NP_BF16)
    in_maps = [
        {"x": _swizzle(xb[c * BLOC : (c + 1) * BLOC]), "gw": gwv}
        for c in range(NCORES)
    ]
    res = run_bass_kernel_spmd(nc, in_maps, core_ids=list(range(NCORES)), **kwargs)
    out = np.concatenate(
        [
            _unswizzle(np.asarray(res.results[c]["out"])).astype(np.float32)
            for c in range(NCORES)
        ],
        axis=0,
    )
    return out, res


def kernel(x):
    return _run(x)[0]
